# revision 17
# baseline (speedup 1.0000x reference)
"""Trainium2 Bass kernel for CurvatureLoss3D.

Input phi [2,1,192,192,192] f32 -> scalar loss.

Sharding: 8 cores = (batch n in {0,1}) x (depth quarter). Each core gets an
input slab [50,192,192] (depth halo included) and computes per-(h,d)-row
partial sums of pen*mask and mask over its 48 output depth rows. Host trims
edge/overlap rows and finishes the scalar reduction.

On-chip layout: partitions = H, free = (shift s, D, W) where the DMA loads
three H-shifted replicas X3[p,s,d,w] = x(d, h0+p+s, w) via an overlapping
access pattern. Compute engines cannot address partition offsets that are
not 32-aligned, so every H-direction stencil tap becomes a free-dim offset
of s*SB instead. Zero-crossing mask via sign-sum (27 neighbors all same
sign <=> |sum sign| == 27). Reciprocals via Ln/Exp with exact EPS placement
(ACT Reciprocal is banned for accuracy).
"""

import os
import sys

sys.path.insert(0, "/opt/trn_rl_repo")

import numpy as np

import bass_rust
import concourse.bass as bass
import concourse.tile as tile
from concourse import bacc
from concourse import mybir
from concourse.bass_utils import run_bass_kernel_spmd

F32 = mybir.dt.float32
BF16 = mybir.dt.bfloat16
ALU = mybir.AluOpType
ACTF = mybir.ActivationFunctionType
AX = mybir.AxisListType

EPS = 1e-8
THETA = 0.5 + 1e-8
INV_THETA = 1.0 / THETA

N = 2
DVOL = 192
W = 192
DOUT = 190          # valid conv output extent per axis
D_IN = 50           # input slab depth rows per core
D_OUT_CORE = 48     # output depth rows computed per core
DB = 6              # output d rows per subblock
NSUB = D_OUT_CORE // DB
FD = DB * W         # pointwise free-dim extent
ROW = 3 * W         # one interleaved d-row in X3: shifts s=0,1,2 concatenated
X3W = (DB + 2) * ROW  # data cols in X3
X3PAD = X3W + 2     # +2 pad cols so trailing w+2 reads stay in-bounds
U3E = DB * ROW + 2  # U3 extent (mirror layout, d 0..DB-1 + 2-col spill)
# (h0, Ph, valid_out_rows)
HBLOCKS = ((0, 128, 126), (126, 64, 64))

# per-core input-slab depth starts; output rows covered = d0..d0+47
CORE_D0 = [0, 48, 96, 142]

_last_results = None  # test harness reads exec time from here


def xo(s, d, w):
    return d * ROW + s * W + w


def _emit(tc, x, outp, outc, dbg=None):
    nc = tc.nc
    import contextlib

    with contextlib.ExitStack() as ctx:
        xpool = ctx.enter_context(tc.tile_pool(name="xin", bufs=2))
        mpool = ctx.enter_context(tc.tile_pool(name="main", bufs=1))
        apool = ctx.enter_context(tc.tile_pool(name="acc", bufs=1))

        accP = apool.tile([128, 2 * D_OUT_CORE], F32, tag="accP", name="accP")
        accC = apool.tile([128, 2 * D_OUT_CORE], F32, tag="accC", name="accC")
        nc.vector.memset(accP[:], 0.0)
        nc.vector.memset(accC[:], 0.0)

        # bias constants for ACT (only 0.0/1.0 are pre-registered)
        bias_tiles = {}
        for i, bval in enumerate((4.0 * EPS, EPS, -1.0)):
            bt = apool.tile([128, 1], F32, tag=f"bias{i}", name=f"bias{i}")
            nc.gpsimd.memset(bt[:], bval)
            bias_tiles[bval] = bt

        def BIAS(v):
            return bias_tiles[v][:, :]

        def T(tag, fd=FD, dt=F32):
            return mpool.tile([128, fd], dt, tag=tag, name=tag)

        TT = nc.vector.tensor_tensor
        STT = nc.vector.scalar_tensor_tensor
        ACT = nc.scalar.activation

        for hb, (h0, ph, _hval) in enumerate(HBLOCKS):
            for j in range(NSUB):
                def DUMP(nm, t):
                    if dbg is not None and hb == 0 and j == 0 and nm in dbg:
                        nc.gpsimd.dma_start(dbg[nm], t)
                din0 = DB * j
                X3 = xpool.tile([128, X3PAD], F32, tag="X3", name="X3")
                src = x.copy()
                src.offset = din0 * DVOL * W + h0 * W
                src.ap = bass_rust.VecI64Pair(
                    [[W, ph], [DVOL * W, DB + 2], [1, ROW]]
                )
                nc.sync.dma_start(X3[0:ph, 0:X3W], src)
                nc.gpsimd.memset(X3[:, X3W:X3PAD], 1.0)

                def xv(s, d, w, n=W, dcnt=DB):
                    # X3 view [dcnt, n] at (s, d, w); d stride = ROW
                    return _view2(X3, xo(s, d, w), ROW, dcnt, n)

                # ---- stencil fields ----
                # U3 mirrors X3 layout: U3[(d,s,w)] = x(d+2,h+s,w) - x(d,h+s,w)
                U3 = T("U3", U3E)
                TT(U3[:, 0:U3E], X3[:, 2 * ROW : 2 * ROW + U3E], X3[:, 0:U3E],
                   ALU.subtract)

                DUMP("U3", U3[:, 0:U3E])

                def uv(s, d, w, n=W):
                    return _view2(U3, xo(s, d, w), ROW, DB, n)

                # V dense rows of 194: V[d*194+w] = x(d+1,h+2,w)-x(d+1,h,w)
                V = T("V", DB * 194)
                TT(_view2(V, 0, 194, DB, 194),
                   _view2(X3, xo(2, 1, 0), ROW, DB, 194),
                   _view2(X3, xo(0, 1, 0), ROW, DB, 194), ALU.subtract)

                DUMP("V", V[:, :])

                def vv(w, n=W):
                    return _view2(V, w, 194, DB, n)

                def dn(t, w=0, n=W):
                    # dense [d][192] tile view
                    return _view2(t, w, W, DB, n)

                t1 = T("t1")
                TT(dn(t1), xv(1, 0, 1), xv(1, 2, 1), ALU.add)
                A = T("A")  # hxx
                STT(dn(A), xv(1, 1, 1), -2.0, dn(t1), ALU.mult, ALU.add)
                t2 = T("t2")
                TT(dn(t2), xv(1, 1, 0), xv(1, 1, 2), ALU.add)
                C0 = T("C0")  # hzz
                STT(dn(C0), xv(1, 1, 1), -2.0, dn(t2), ALU.mult, ALU.add)
                W1 = T("W1")  # 2gz
                TT(dn(W1), xv(1, 1, 2), xv(1, 1, 0), ALU.subtract)
                t3 = T("t1")
                TT(dn(t3), xv(0, 1, 1), xv(2, 1, 1), ALU.add)
                B = T("B")  # hyy
                STT(dn(B), xv(1, 1, 1), -2.0, dn(t3), ALU.mult, ALU.add)
                DUMP("A", A[:, :]); DUMP("B", B[:, :])
                P = T("P")  # 4hxy
                TT(dn(P), uv(2, 0, 1), uv(0, 0, 1), ALU.subtract)
                Q = T("Q")  # 4hxz
                TT(dn(Q), uv(1, 0, 2), uv(1, 0, 0), ALU.subtract)
                R = T("R")  # 4hyz
                TT(dn(R), vv(2), vv(0), ALU.subtract)

                # ---- squares (ACT) ----
                U2 = T("U2")
                ACT(dn(U2), uv(1, 0, 1), ACTF.Square)
                V2 = T("V2")
                ACT(dn(V2), vv(1), ACTF.Square)
                W2s = T("W2s")
                ACT(W2s[:, :], W1[:, :], ACTF.Square)

                # ---- S2 = 4|g|^2 and the Ln/Exp cluster ----
                S2 = T("S2")
                TT(S2[:, :], U2[:, :], V2[:, :], ALU.add)
                TT(S2[:, :], S2[:, :], W2s[:, :], ALU.add)
                DUMP("S2", S2[:, :])
                L = T("cL")
                ACT(L[:, :], S2[:, :], ACTF.Ln, bias=BIAS(4.0 * EPS))
                rt = T("cA")  # 2mag
                ACT(rt[:, :], L[:, :], ACTF.Exp, scale=0.5)
                D3 = T("cB")  # 8mag^3
                ACT(D3[:, :], L[:, :], ACTF.Exp, scale=1.5)
                LD = T("cC")
                ACT(LD[:, :], D3[:, :], ACTF.Ln, bias=BIAS(EPS), scale=0.125)
                R3 = T("cB")  # 1/(mag^3+EPS)
                ACT(R3[:, :], LD[:, :], ACTF.Exp, scale=-1.0)
                LR = T("cC")
                ACT(LR[:, :], rt[:, :], ACTF.Ln, bias=BIAS(EPS), scale=0.5)
                R1 = T("cL")  # 1/(mag+EPS)
                ACT(R1[:, :], LR[:, :], ACTF.Exp, scale=-1.0)
                DUMP("R3", R3[:, :]); DUMP("R1", R1[:, :])

                # ---- trace and F = 4*g^T H g ----
                trH = T("trH")
                TT(trH[:, :], A[:, :], B[:, :], ALU.add)
                TT(trH[:, :], trH[:, :], C0[:, :], ALU.add)

                UVt = T("s0")
                TT(dn(UVt), uv(1, 0, 1), vv(1), ALU.mult)
                F1 = T("s1")
                TT(F1[:, :], UVt[:, :], P[:, :], ALU.mult)
                TT(dn(UVt), uv(1, 0, 1), dn(W1), ALU.mult)
                F2 = T("s2")
                TT(F2[:, :], UVt[:, :], Q[:, :], ALU.mult)
                TT(F1[:, :], F1[:, :], F2[:, :], ALU.add)
                TT(dn(UVt), vv(1), dn(W1), ALU.mult)
                TT(UVt[:, :], UVt[:, :], R[:, :], ALU.mult)
                TT(F1[:, :], F1[:, :], UVt[:, :], ALU.add)  # Fc

                Fd = T("s3")
                TT(Fd[:, :], U2[:, :], A[:, :], ALU.mult)
                TT(F2[:, :], V2[:, :], B[:, :], ALU.mult)
                TT(Fd[:, :], Fd[:, :], F2[:, :], ALU.add)
                TT(F2[:, :], W2s[:, :], C0[:, :], ALU.mult)
                TT(Fd[:, :], Fd[:, :], F2[:, :], ALU.add)
                Ff = T("s0")  # F = Fd + 0.5*Fc
                STT(Ff[:, :], F1[:, :], 0.5, Fd[:, :], ALU.mult, ALU.add)

                # ---- curvature glue ----
                G = T("s1")
                TT(G[:, :], S2[:, :], trH[:, :], ALU.mult)
                TT(G[:, :], G[:, :], Ff[:, :], ALU.subtract)  # 4*NM
                mc = T("s2")
                STT(mc[:, :], G[:, :], 0.25, R3[:, :], ALU.mult, ALU.mult)
                qd = T("s3")
                STT(qd[:, :], Ff[:, :], 0.25, R3[:, :], ALU.mult, ALU.mult)
                lap = T("s0")
                TT(lap[:, :], trH[:, :], R1[:, :], ALU.mult)
                TT(lap[:, :], lap[:, :], qd[:, :], ALU.subtract)  # gauss
                mc2 = T("s1")
                ACT(mc2[:, :], mc[:, :], ACTF.Square)
                TT(mc2[:, :], mc2[:, :], lap[:, :], ALU.subtract)  # dq
                ACT(mc2[:, :], mc2[:, :], ACTF.Abs)  # |dq|
                LQ = T("s0")
                ACT(LQ[:, :], mc2[:, :], ACTF.Ln, bias=BIAS(EPS))
                sqv = T("s1")
                ACT(sqv[:, :], LQ[:, :], ACTF.Exp, scale=0.5)
                k1 = T("s0")
                TT(k1[:, :], mc[:, :], sqv[:, :], ALU.add)
                k2 = T("s1")
                ACT(k2[:, :], k1[:, :], ACTF.Square, scale=INV_THETA)
                pen = T("s0")
                ACT(pen[:, :], k2[:, :], ACTF.Relu, bias=BIAS(-1.0))
                DUMP("pen", pen[:, :])

                # ---- zero-crossing mask via sign sums (bf16) ----
                sgn = T("sgn", X3PAD, BF16)
                ACT(sgn[:, :], X3[:, :], ACTF.Sign)
                sw = T("sw", X3W, BF16)
                TT(sw[:, 0:X3W], sgn[:, 0:X3W], sgn[:, 1 : 1 + X3W], ALU.add)
                TT(sw[:, 0:X3W], sw[:, 0:X3W], sgn[:, 2 : 2 + X3W], ALU.add)
                sh = T("sh", 8 * W, BF16)
                TT(_view2(sh, 0, W, 8, W), _view2(sw, 0, ROW, 8, W),
                   _view2(sw, W, ROW, 8, W), ALU.add)
                TT(_view2(sh, 0, W, 8, W), _view2(sh, 0, W, 8, W),
                   _view2(sw, 2 * W, ROW, 8, W), ALU.add)
                sd = T("sd", FD, BF16)
                TT(sd[:, :], sh[:, 0:FD], sh[:, W : W + FD], ALU.add)
                TT(sd[:, :], sd[:, :], sh[:, 2 * W : 2 * W + FD], ALU.add)
                TT(sd[:, :], sd[:, :], sd[:, :], ALU.mult)
                DUMP("sd", sd[:, :])
                mask = T("mask", FD, BF16)
                nc.vector.tensor_single_scalar(mask[:, :], sd[:, :], 729.0, ALU.is_lt)
                m3 = mask[:, :].rearrange("p (d w) -> p d w", w=W)
                nc.gpsimd.memset(m3[:, :, DOUT:W], 0.0)
                DUMP("mask", mask[:, :])

                # ---- masked penalty + per-d-row reductions ----
                penm = T("s1")
                TT(penm[:, :], pen[:, :], mask[:, :], ALU.mult)
                col = hb * D_OUT_CORE + DB * j
                nc.vector.tensor_reduce(
                    accP[:, col : col + DB],
                    penm[:, :].rearrange("p (d w) -> p d w", w=W),
                    AX.X,
                    ALU.add,
                )
                nc.vector.tensor_reduce(
                    accC[:, col : col + DB],
                    mask[:, :].rearrange("p (d w) -> p d w", w=W),
                    AX.X,
                    ALU.add,
                )

        nc.sync.dma_start(outp, accP[:, :].rearrange("p (b d) -> p b d", b=2))
        nc.sync.dma_start(outc, accC[:, :].rearrange("p (b d) -> p b d", b=2))


def _install_ntff_hook_shim():
    """Recreate antenv.axon_hooks (absent in this image) so trace=True works."""
    import sys as _sys
    import types
    if "antenv.axon_hooks" in _sys.modules:
        return
    try:
        from trn_agent_boot.trn_boot import _ntff_profile_via_ctypes
        hook = _ntff_profile_via_ctypes("/opt/axon/libaxon_pjrt.so")
    except Exception as e:
        print("ntff shim failed:", e)
        hook = None
    mod = types.ModuleType("antenv.axon_hooks")
    _state = {"hook": hook}
    mod.get_axon_ntff_profile_hook = lambda: _state["hook"]
    mod.set_axon_ntff_profile_hook = lambda h: _state.update(hook=h)
    _sys.modules["antenv.axon_hooks"] = mod
    import antenv
    antenv.axon_hooks = mod


def _view2(t, off, dstep, dcnt, n):
    """AP view of tile t: all partitions, free dims [(dstep, dcnt), (1, n)] at off."""
    ap = t[:, 0:1].copy()
    base = ap.ap.to_list()
    pdim = base[0]
    ap.offset = ap.offset + off
    ap.ap = bass_rust.VecI64Pair([list(pdim), [dstep, dcnt], [1, n]])
    return ap


def _build_nc():
    nc = bacc.Bacc("TRN2", target_bir_lowering=False, debug=False, num_devices=8)
    x = nc.dram_tensor("x", [D_IN, DVOL, W], F32, kind="ExternalInput")
    outp = nc.dram_tensor("outp", [128, 2, D_OUT_CORE], F32, kind="ExternalOutput")
    outc = nc.dram_tensor("outc", [128, 2, D_OUT_CORE], F32, kind="ExternalOutput")
    with tile.TileContext(nc) as tc:
        _emit(tc, x.ap(), outp.ap(), outc.ap())
    nc.finalize()
    return nc


def kernel(phi):
    global _last_results
    phi = np.asarray(phi)
    assert phi.shape == (N, 1, DVOL, DVOL, W), phi.shape
    nc = _build_nc()
    in_maps = []
    for c in range(8):
        n, q = divmod(c, 4)
        d0 = CORE_D0[q]
        slab = np.ascontiguousarray(phi[n, 0, d0 : d0 + D_IN], dtype=np.float32)
        in_maps.append({"x": slab})
    trace = bool(int(os.environ.get("KERNEL_TRACE", "0")))
    if trace:
        _install_ntff_hook_shim()
    res = run_bass_kernel_spmd(nc, in_maps, list(range(8)), trace=trace)
    _last_results = res
    tp = 0.0
    tcnt = 0.0
    for c in range(8):
        op = res.results[c]["outp"].astype(np.float64)
        oc = res.results[c]["outc"].astype(np.float64)
        dlo = 2 if (c % 4) == 3 else 0
        for hb, (_h0, _ph, hval) in enumerate(HBLOCKS):
            tp += op[:hval, hb, dlo:].sum()
            tcnt += oc[:hval, hb, dlo:].sum()
    return np.float32(tp / (tcnt + EPS))
